# revision 20
# baseline (speedup 1.0000x reference)
"""Trainium2 Bass kernel for nn_DemandTemporalEncoder.

TCN (6 dilated causal conv blocks) + sparse top-p attention, data-parallel
over batch across 8 NeuronCores (1 batch sample per core).

Key algebraic facts used:
  * Only attn_out[:, -1, :] is consumed, so attention needs just one query
    (the last position): a single score row s[t] = q . k_t / sqrt(D).
  * s = (Wk^T q) . z_t + q.bk ; the constant q.bk shifts every score equally
    and cancels in both top-k selection and softmax, so K is never built.
  * top-512-of-2048 is computed exactly via rank counting:
    rank_i = #{j : s_j > s_i}; keep rank < 512 (ties have measure zero).
  * exp without max-subtraction: scores are O(0.1), and softmax is
    shift-invariant so the result matches the reference's stabilized form.
  * The 11 KS=3 convs (b0 conv2 + 5 blocks x 2) use Winograd F(2,3):
    for output pair (y[t], y[t+d]) with taps (g0,g1,g2) on x[t-2d..t]:
      E[t']   = x[t'-2d] - x[t']  (one full-length stream; even blocks of d
                give D1 = E[t], odd blocks give D4 = E[t+d])
      P2[tau] = x[t-d] + x[t],  P3[tau] = x[t] - x[t-d]   (even blocks)
      m1 = G1@E_even, m2 = G2@P2, m3 = G3@P3, m4 = G4@E_odd
      with G1 = g0, G2 = (g0+g1+g2)/2, G3 = (g0-g1+g2)/2, G4 = g2
      y[t] = m1+m2+m3,  y[t+d] = m2-m3-m4
    4 matmul-columns per 2 outputs instead of 6 (-33% PE). Transforms run
    on gpsimd (SBUF-only; it has no PSUM access), combines on DVE, bias+gelu
    on ACT with block-interleaved destination APs.

Layouts (per core):
  * Activations channel-major in SBUF: [128 part = channel-in-chunk,
    4 chunks x (64 pad + 2048 t)] so a dilated causal shift is a column
    offset and the zero left-pad implements causal padding (PAD = 64 = 2*32
    exactly covers the deepest dilation's x[t-2d] reach).
  * Conv weights and activations are bf16 (fp32 PSUM accumulation); the
    score row and the softmax weights stay 32-bit. Host-side weight prep
    also folds Wu = Wk^T Wq and Wpv = Wp Wv so the attention tail is two
    matvecs, one score row, a rank scan (split across DVE and ACT), and one
    w @ V' pass.
"""

import sys

if '/opt/trn_rl_repo' not in sys.path:
    sys.path.insert(0, '/opt/trn_rl_repo')

import numpy as np

B, T, D_IN, D, KS = 8, 2048, 64, 512, 3
N_LAYERS = 6
PAD = 64            # max dilation (32) * (KS-1)
CT = PAD + T        # padded time extent per channel chunk
NCH = 4             # 512 / 128 channel chunks
NTT = 4             # time tiles of 512 for matmul free dim
NTC = 16            # time chunks of 128 for attention
TAU = T // 2        # winograd tile count per channel
K_KEEP = 512        # int(0.25 * T)
SQRT_D_INV = 1.0 / float(np.sqrt(np.float32(D)))

_CACHE = {}


def _build_program(debug_taps=False):
    import concourse.tile as tile
    from concourse import bacc, mybir
    from contextlib import ExitStack

    F32 = mybir.dt.float32
    F32R = mybir.dt.float32r
    BF16 = mybir.dt.bfloat16
    AF = mybir.ActivationFunctionType
    ALU = mybir.AluOpType

    nc = bacc.Bacc("TRN2", target_bir_lowering=False, debug=False, num_devices=8)

    xcm_d = nc.dram_tensor("xcm", [128, CT], BF16, kind="ExternalInput")
    w0c1_d = nc.dram_tensor("w0c1", [128, 2 * D], BF16, kind="ExternalInput")
    wres_d = nc.dram_tensor("wres", [D_IN, D], BF16, kind="ExternalInput")
    wmain_d = nc.dram_tensor("wmain", [11, 128, 16 * D], BF16, kind="ExternalInput")
    wu_d = nc.dram_tensor("wu", [128, NCH * D], BF16, kind="ExternalInput")
    wpv_d = nc.dram_tensor("wpv", [128, NCH * D], BF16, kind="ExternalInput")
    wp_d = nc.dram_tensor("wp", [128, NCH * D], F32, kind="ExternalInput")
    bcol_d = nc.dram_tensor("bcol", [128, 17 * NCH], F32, kind="ExternalInput")
    ones_d = nc.dram_tensor("ones", [1, 128], F32R, kind="ExternalInput")
    zpad_d = nc.dram_tensor("zpad", [128, PAD], BF16, kind="ExternalInput")
    out_d = nc.dram_tensor("out", [D], F32, kind="ExternalOutput")
    dbg = {}
    if debug_taps:
        for nm, shp, dt_ in [("dbg_y1b0", [128, NCH * CT], BF16),
                             ("dbg_h0", [128, NCH * CT], BF16),
                             ("dbg_z", [128, NCH * CT], BF16),
                             ("dbg_srow", [1, T], F32)]:
            dbg[nm] = nc.dram_tensor(nm, shp, dt_, kind="ExternalOutput")

    def r(ap):
        return ap

    def f32(ap):
        return ap.bitcast(F32)

    with ExitStack() as ctx:
        tc = ctx.enter_context(tile.TileContext(nc))
        const = ctx.enter_context(tc.tile_pool(name="const", bufs=1))
        wpool = ctx.enter_context(tc.tile_pool(name="w", bufs=2))
        hpool = ctx.enter_context(tc.tile_pool(name="h", bufs=1))
        ypool = ctx.enter_context(tc.tile_pool(name="y", bufs=1))
        tpool = ctx.enter_context(tc.tile_pool(name="tf", bufs=2))
        epool = ctx.enter_context(tc.tile_pool(name="e", bufs=2))
        spool = ctx.enter_context(tc.tile_pool(name="s", bufs=1))
        psacc = ctx.enter_context(tc.tile_pool(name="psacc", bufs=4, space="PSUM"))
        psaux = ctx.enter_context(tc.tile_pool(name="psaux", bufs=4, space="PSUM"))
        dpool = ctx.enter_context(tc.tile_pool(name="dram", bufs=1, space="DRAM"))

        xsb = const.tile([128, CT], BF16, tag="x")
        nc.sync.dma_start(xsb[:, 0:PAD + 1024], xcm_d.ap()[:, 0:PAD + 1024])
        nc.sync.dma_start(xsb[:, PAD + 1024:CT], xcm_d.ap()[:, PAD + 1024:CT])
        w0c1 = const.tile([128, 2 * D], BF16, tag="w0c1")
        nc.sync.dma_start(w0c1[:], w0c1_d.ap()[:])
        bcol = const.tile([128, 17 * NCH], F32, tag="bcol")
        nc.scalar.dma_start(bcol[:], bcol_d.ap()[:])
        wres = const.tile([D_IN, D], BF16, tag="wres")
        nc.scalar.dma_start(wres[:], wres_d.ap()[:])
        ones1 = const.tile([1, 128], F32R, tag="ones1")
        nc.scalar.dma_start(ones1[:], ones_d.ap()[:])
        ones128 = const.tile([128, 1], F32, tag="ones128")
        nc.vector.memset(ones128[:], 1.0)

        h = hpool.tile([128, NCH * CT], BF16, tag="h")
        y1 = ypool.tile([128, NCH * CT], BF16, tag="y")
        for cc in range(NCH):
            nc.scalar.dma_start(h[:, cc * CT:cc * CT + PAD], zpad_d.ap()[:])
            nc.scalar.dma_start(y1[:, cc * CT:cc * CT + PAD], zpad_d.ap()[:])

        def bias_ap(vi, mo):
            return bcol[:, vi * NCH + mo:vi * NCH + mo + 1]

        # ------- block 0 conv1: taps (t, t-1) packed on 128 partitions -------
        # xsb: part 0-63 = x(t), part 64-127 = x(t-1); tap t-2 reads part
        # 0-63 at column offset -2. Two matmuls per 512-col psum group.
        for tt in range(NTT):
            for mo in range(NCH):
                pt = psacc.tile([128, 512], F32, tag="acc", name=f"c1_{tt}_{mo}")
                a = PAD + tt * 512
                nc.tensor.matmul(pt[:], w0c1[:, mo * 128:mo * 128 + 128],
                                 xsb[:, a:a + 512], start=True, stop=False)
                nc.tensor.matmul(pt[:], w0c1[0:D_IN, D + mo * 128:D + mo * 128 + 128],
                                 xsb[0:D_IN, a - 2:a + 510], start=False, stop=True)
                nc.scalar.activation(y1[:, mo * CT + a:mo * CT + a + 512],
                                     pt[:], AF.Gelu, bias=bias_ap(0, mo))

        if debug_taps:
            nc.sync.dma_start(dbg["dbg_y1b0"].ap()[:], y1[:])

        # ---- 1x1 residual -> h (h then gets += gelu(conv2) via winograd) ----
        # attention weight packs: needed ~400us in; stream them during convs
        packA = const.tile([128, NCH * D], BF16, tag="packa")
        nc.sync.dma_start(packA[:], wu_d.ap()[:])
        packV = const.tile([128, NCH * D], BF16, tag="packv")
        nc.sync.dma_start(packV[:], wpv_d.ap()[:])
        packP = const.tile([128, NCH * D], F32, tag="packp")
        nc.sync.dma_start(packP[:], wp_d.ap()[:])
        for mo in reversed(range(NCH)):
            for tt in range(NTT):
                pr = psaux.tile([128, 512], F32, tag="aux")
                nc.tensor.matmul(pr[:], r(wres[:, mo * 128:mo * 128 + 128]),
                                 r(xsb[0:D_IN, PAD + tt * 512:PAD + tt * 512 + 512]),
                                 start=True, stop=True)
                nc.scalar.activation(h[:, mo * CT + PAD + tt * 512:mo * CT + PAD + tt * 512 + 512],
                                     pr[:], AF.Identity, bias=bias_ap(2, mo))

        # ---------------- winograd F(2,3) convs ----------------
        def wino_conv(ci, dil, src, dst, to_h, vi):
            """One KS=3 causal conv: dst <- gelu(conv(src)+bias) (to_h=False)
            or dst <- dst + gelu(conv(src)+bias) (to_h=True)."""
            wsb = wpool.tile([128, 16 * D], BF16, tag="w")
            nc.sync.dma_start(wsb[:], wmain_d.ap()[ci])
            E = tpool.tile([128, NCH * T], BF16, tag="E")
            P2 = tpool.tile([128, NCH * TAU], BF16, tag="P2")
            P3 = tpool.tile([128, NCH * TAU], BF16, tag="P3")
            for cc in reversed(range(NCH)):
                s0 = cc * CT + PAD

                def ev(q):
                    return src[:, s0 + q:s0 + q + T].rearrange(
                        "p (b o) -> p b o", o=2 * dil)[:, :, 0:dil]

                def pout(Pt):
                    return Pt[:, cc * TAU:(cc + 1) * TAU].rearrange(
                        "p (b o) -> p b o", o=dil)

                nc.vector.tensor_tensor(E[:, cc * T:(cc + 1) * T],
                                        src[:, s0 - 2 * dil:s0 - 2 * dil + T],
                                        src[:, s0:s0 + T], op=ALU.subtract)
                nc.vector.tensor_tensor(pout(P2), ev(-dil), ev(0), op=ALU.add)
                nc.vector.tensor_tensor(pout(P3), ev(0), ev(-dil), op=ALU.subtract)

            def rhs_view(j, cc, th):
                if j in (0, 3):
                    lo = 0 if j == 0 else dil
                    return E[:, cc * T + th * 1024:cc * T + th * 1024 + 1024].rearrange(
                        "p (b o) -> p b o", o=2 * dil)[:, :, lo:lo + dil]
                Pt = P2 if j == 1 else P3
                return Pt[:, cc * TAU + th * 512:cc * TAU + th * 512 + 512]

            for mo in reversed(range(NCH)):
                for th in range(2):
                    pool_ = psacc if th == 0 else psaux
                    tg = "acc" if th == 0 else "aux"
                    banks = [pool_.tile([128, 512], F32, tag=tg, name=f"m{ci}_{mo}_{j}_{th}")
                             for j in range(4)]
                    # cc descending outermost: 12 of 16 matmuls consume chunks
                    # finished early, covering the latest chunk's transform lag
                    for cc in reversed(range(NCH)):
                        for j in range(4):
                            cbase = ((cc * 4 + j) * NCH + mo) * 128
                            lhsT = wsb[:, cbase:cbase + 128]
                            nc.tensor.matmul(banks[j][:], lhsT, rhs_view(j, cc, th),
                                             start=(cc == NCH - 1), stop=(cc == 0))
                    m1, m2, m3, m4 = (bk[:] for bk in banks)
                    # DVE may read only one PSUM operand per op: stage m2 in SBUF
                    s2 = epool.tile([128, 512], F32, tag="c0")
                    nc.scalar.copy(s2[:], m2)
                    c0 = epool.tile([128, 512], F32, tag="c1")
                    nc.vector.tensor_add(c0[:], s2[:], m1)
                    y0p = epool.tile([128, 512], F32, tag="c2")
                    nc.vector.tensor_add(y0p[:], c0[:], m3)
                    c2t = epool.tile([128, 512], F32, tag="c3")
                    nc.vector.tensor_tensor(c2t[:], s2[:], m3, op=ALU.subtract)
                    y1p = epool.tile([128, 512], F32, tag="c4")
                    nc.vector.tensor_tensor(y1p[:], c2t[:], m4, op=ALU.subtract)
                    base = mo * CT + PAD + th * 1024

                    def dv(lo):
                        return dst[:, base:base + 1024].rearrange(
                            "p (b o) -> p b o", o=2 * dil)[:, :, lo:lo + dil]

                    if not to_h:
                        nc.scalar.activation(dv(0), y0p[:], AF.Gelu, bias=bias_ap(vi, mo))
                        nc.scalar.activation(dv(dil), y1p[:], AF.Gelu, bias=bias_ap(vi, mo))
                    else:
                        te = epool.tile([128, 512], BF16, tag="g0")
                        nc.scalar.activation(te[:], y0p[:], AF.Gelu, bias=bias_ap(vi, mo))
                        nc.gpsimd.tensor_add(dv(0), dv(0), te[:])
                        to_ = epool.tile([128, 512], BF16, tag="g1")
                        nc.scalar.activation(to_[:], y1p[:], AF.Gelu, bias=bias_ap(vi, mo))
                        nc.gpsimd.tensor_add(dv(dil), dv(dil), to_[:])

        # block 0 conv2 (dil=1), residual already in h
        wino_conv(0, 1, y1, h, True, 1)
        if debug_taps:
            nc.sync.dma_start(dbg["dbg_h0"].ap()[:], h[:])
        # blocks 1..5, identity residual
        for i in range(N_LAYERS - 1):
            dil = 2 ** (i + 1)
            wino_conv(1 + 2 * i, dil, h, y1, False, 3 + 2 * i)
            wino_conv(2 + 2 * i, dil, y1, h, True, 4 + 2 * i)

        if debug_taps:
            nc.sync.dma_start(dbg["dbg_z"].ap()[:], h[:])

        # ---------------- attention (z = h) ----------------
        zlast = spool.tile([128, NCH], BF16, tag="zlast")
        zl_src = h[:].rearrange("p (c t) -> p c t", c=NCH)[:, :, PAD + T - 1]
        nc.vector.tensor_copy(zlast[:], zl_src)
        # zlb = z_last + bv (bv folds out of the attention sum: weights sum to 1)
        zlb = spool.tile([128, NCH], F32, tag="zlb")
        nc.vector.tensor_add(zlb[:], zlast[:], bcol[:, 15 * NCH:16 * NCH])

        # u = Wu z_last + bu, Wu = Wk^T Wq, bu = Wk^T bq (host-folded).
        # The q.bk constant cancels in softmax/top-k.
        pu = psaux.tile([128, NCH], F32, tag="aux")
        for mc in range(NCH):
            for cc in range(NCH):
                nc.tensor.matmul(pu[:, mc:mc + 1], packA[:, cc * D + mc * 128:cc * D + mc * 128 + 128],
                                 zlast[:, cc:cc + 1], start=(cc == 0), stop=(cc == NCH - 1))
        ucol = spool.tile([128, NCH], BF16, tag="ucol")
        nc.vector.tensor_add(ucol[:], pu[:], bcol[:, 13 * NCH:14 * NCH])

        # s[t] = u . z_t / sqrt(D)   (row layout [1, 2048])
        srow = spool.tile([1, T], F32R, tag="srow")
        for tt in range(NTT):
            psm = psaux.tile([1, 512], F32, tag="aux")
            for cc in range(NCH):
                nc.tensor.matmul(psm[:], r(ucol[:, cc:cc + 1]),
                                 r(h[:, cc * CT + PAD + tt * 512:cc * CT + PAD + tt * 512 + 512]),
                                 start=(cc == 0), stop=(cc == NCH - 1))
            nc.scalar.mul(srow[0:1, tt * 512:tt * 512 + 512], psm[:], SQRT_D_INV)

        # s in [128, 16] layout: (p, ci) = s[ci*128 + p].
        # Bounced through DRAM per time-tile so the transpose pipelines
        # behind the s-stage instead of serializing after it.
        sbounce = dpool.tile([1, T], F32, tag="sbounce")
        spt = spool.tile([128, NTC], F32, tag="spt")
        for tt in range(NTT):
            nc.gpsimd.dma_start(sbounce[0:1, tt * 512:tt * 512 + 512],
                                f32(srow[0:1, tt * 512:tt * 512 + 512]))
            nc.gpsimd.dma_start(
                spt[:, tt * NCH:(tt + 1) * NCH],
                sbounce[0:1, tt * 512:tt * 512 + 512].rearrange("a (b c) -> (a c) b", b=NCH))

        # broadcast s to all partitions via K=1 outer-product matmul.
        # bf16 copy: ranking bf16-rounded scores keeps the scan in DVE 2x
        # mode; near-tie flips only swap ~equal softmax weights (validated).
        sbcast = spool.tile([128, T], BF16, tag="sbcast")
        for tt in range(NTT):
            pb = psacc.tile([128, 512], F32, tag="acc")
            nc.tensor.matmul(pb[:], r(ones1[:]), r(srow[0:1, tt * 512:tt * 512 + 512]),
                             start=True, stop=True)
            nc.scalar.copy(sbcast[:, tt * 512:tt * 512 + 512], pb[:])
        sptb = spool.tile([128, NTC], BF16, tag="sptb")
        nc.vector.tensor_copy(sptb[:], spt[:])

        # V' tiles: V'_tm[ci] = (z_chunk)^T @ (Wp Wv)^T -> [t 128, d 512]
        # (Wpv host-folded; evacuations split DVE/ACT to balance the tail)
        vtm = ypool.tile([128, NTC * D], BF16, tag="y")
        vps = []
        for ci in range(NTC):
            pv = psacc.tile([128, 512], F32, tag="acc", name=f"vps{ci}")
            for cc in range(NCH):
                nc.tensor.matmul(pv[:], r(h[:, cc * CT + PAD + ci * 128:cc * CT + PAD + ci * 128 + 128]),
                                 packV[:, cc * D:cc * D + D],
                                 start=(cc == 0), stop=(cc == NCH - 1))
            vps.append(pv)

        def v_evac(ci, eng):
            eng(vtm[:, ci * D:ci * D + D], vps[ci][:])

        # exp(s) does not depend on the rank scan: compute it up front
        ept = spool.tile([128, NTC], F32, tag="ept")
        nc.scalar.activation(ept[:], spt[:], AF.Exp)
        negspt = spool.tile([128, 6], F32, tag="negspt")
        nc.vector.tensor_scalar_mul(negspt[:], sptb[:, 10:NTC], -1.0)

        # pre = Wp @ (z_last + bv) + bp, bounced to row layout off-critical-path
        ppre = psaux.tile([128, NCH], F32, tag="aux")
        for mo in range(NCH):
            for cc in range(NCH):
                nc.tensor.matmul(ppre[:, mo:mo + 1], packP[:, cc * D + mo * 128:cc * D + mo * 128 + 128],
                                 zlb[:, cc:cc + 1], start=(cc == 0), stop=(cc == NCH - 1))
        pre = spool.tile([128, NCH], F32, tag="pre")
        nc.vector.tensor_add(pre[:], ppre[:], bcol[:, 16 * NCH:17 * NCH])
        pbounce = dpool.tile([128, NCH], F32, tag="pbounce")
        nc.gpsimd.dma_start(pbounce[:], pre[:])
        prerow = spool.tile([1, D], F32, tag="prerow")
        nc.gpsimd.dma_start(prerow[:], pbounce[:].rearrange("a b -> b a"))

        # V' evacuations on ACT (DVE is loaded with the rank scan here)
        for ci in range(NTC):
            v_evac(ci, lambda o, i: nc.scalar.copy(o, i))

        # rank_i = #{j : s_j > s_i}: 10 sweeps on DVE (is_gt, bf16 2x) + 6 on
        # ACT (Sign); bf16 ties half-count in the Sign half, which only
        # perturbs near-threshold picks with ~equal weights.
        junk = spool.tile([128, T], BF16, tag="junk")
        junkA = spool.tile([128, T], BF16, tag="junkA")
        rank = spool.tile([128, NTC], F32, tag="rank")
        sgn = spool.tile([128, 6], F32, tag="sgn")
        for ci in range(10):
            nc.vector.tensor_scalar(junk[:], sbcast[:], spt[:, ci:ci + 1], None,
                                    op0=ALU.is_gt, op1=ALU.add,
                                    accum_out=rank[:, ci:ci + 1])
        for ci in range(6):
            nc.scalar.activation(junkA[:], sbcast[:], AF.Sign,
                                 bias=negspt[:, ci:ci + 1],
                                 accum_out=sgn[:, ci:ci + 1])
        # sum(sign(s_j - s_i)) = #greater - #less; with self(=0): rank = (sgn+2047)/2
        nc.vector.tensor_scalar(rank[:, 10:NTC], sgn[:], float(T - 1), 0.5,
                                op0=ALU.add, op1=ALU.mult)

        # w = exp(s) * (rank < 512)
        keep = spool.tile([128, NTC], F32, tag="keep")
        nc.vector.tensor_scalar(keep[:], rank[:], float(K_KEEP) - 0.5, None, op0=ALU.is_lt)
        wpt = spool.tile([128, NTC], BF16, tag="wpt")
        nc.vector.tensor_mul(wpt[:], ept[:], keep[:])

        # Z = sum(w); 1/Z
        wsum = spool.tile([128, 1], F32, tag="wsum")
        nc.vector.reduce_sum(wsum[:], wpt[:], axis=mybir.AxisListType.X)
        pz = psaux.tile([1, 1], F32, tag="aux")
        nc.tensor.matmul(pz[:], wsum[:], ones128[:], start=True, stop=True)
        rz = spool.tile([1, 1], F32, tag="rz")
        nc.vector.reciprocal(rz[:], pz[:])

        # out = pre + (w @ V') / Z   -- all in row layout on partition 0
        po = psaux.tile([1, 512], F32, tag="aux")
        for ci in range(NTC):
            nc.tensor.matmul(po[:], r(wpt[:, ci:ci + 1]), r(vtm[:, ci * D:ci * D + D]),
                             start=(ci == 0), stop=(ci == NTC - 1))
        outrow = spool.tile([1, D], F32, tag="outrow")
        nc.vector.tensor_scalar(outrow[:], po[:], rz[:], None, op0=ALU.mult)
        nc.vector.tensor_add(outrow[:], outrow[:], prerow[:])
        if debug_taps:
            nc.sync.dma_start(dbg["dbg_srow"].ap()[:], f32(srow[:]))
        nc.sync.dma_start(out_d.ap()[None, :], outrow[:])

    nc.compile()
    return nc


def get_program(debug_taps=False):
    key = 'nc_dbg' if debug_taps else 'nc'
    if key not in _CACHE:
        _CACHE[key] = _build_program(debug_taps)
    return _CACHE[key]


def _pack_chunked(w):
    """[d_out, c_in] (512x512) -> [128, 4*512] with [p, cc*512+m] = w[cc*128+p, m].

    Pass w already oriented so that rows are the matmul contraction dim.
    """
    return np.ascontiguousarray(
        w.reshape(NCH, 128, D).transpose(1, 0, 2).reshape(128, NCH * D))


def _pack_wino(w):
    """[C_out, C_in=512, KS=3] fp32 -> [128, 16*512] bf16 of G_j tiles with
    col ((cc*4 + j)*NCH + mo)*128 + m -> G_j[mo*128+m, cc*128+p]."""
    g0, g1, g2 = w[:, :, 0], w[:, :, 1], w[:, :, 2]
    G = np.stack([g0, (g0 + g1 + g2) * 0.5, (g0 - g1 + g2) * 0.5, g2])  # [4,O,I]
    G = G.reshape(4, NCH, 128, NCH, 128)            # [j, mo, m, cc, p]
    X = G.transpose(4, 3, 0, 1, 2)                  # [p, cc, j, mo, m]
    return np.ascontiguousarray(X.reshape(128, 16 * D))


def _bias_col(v):
    return np.ascontiguousarray(v.reshape(NCH, 128).T)


def make_in_maps(x, c1w0, c1b0, c2w0, c2b0, resw, resb, c1w, c1b, c2w, c2b,
                 wq, bq, wk, bk, wv, bv, wp, bp):
    import ml_dtypes
    bf16 = ml_dtypes.bfloat16
    f = lambda a: np.asarray(a, dtype=np.float32)
    x = f(x)

    # block0 conv1 taps packed: part 0-63 tap@t (w[..,2]), part 64-127
    # tap@t-1 (w[..,1]); second D block, part 0-63: tap@t-2 (w[..,0]).
    w0 = f(c1w0)
    w0c1 = np.zeros((128, 2 * D), np.float32)
    w0c1[0:64, 0:D] = w0[:, :, 2].T
    w0c1[64:128, 0:D] = w0[:, :, 1].T
    w0c1[0:64, D:2 * D] = w0[:, :, 0].T
    w0c1 = w0c1.astype(bf16)
    wres_p = np.ascontiguousarray(f(resw)[:, :, 0].T).astype(bf16)

    convs = [f(c2w0)]
    for i in range(N_LAYERS - 1):
        convs.append(f(c1w)[i])
        convs.append(f(c2w)[i])
    wmain = np.stack([_pack_wino(w) for w in convs]).astype(bf16)

    wu = (f(wk).astype(np.float64).T @ f(wq).astype(np.float64)).astype(np.float32)
    wpv = (f(wp).astype(np.float64) @ f(wv).astype(np.float64)).astype(np.float32)
    wu_p = _pack_chunked(wu.T).astype(bf16)         # lhsT tiles for u = Wu z_last
    wpv_p = _pack_chunked(wpv.T).astype(bf16)       # rhs tiles for V' = (Wp Wv) z
    wp_p = _pack_chunked(f(wp).T)                   # lhsT tiles for pre

    bvecs = [f(c1b0), f(c2b0), f(resb)]
    for i in range(N_LAYERS - 1):
        bvecs.append(f(c1b)[i])
        bvecs.append(f(c2b)[i])
    bu = (f(wk).astype(np.float64).T @ f(bq).astype(np.float64)).astype(np.float32)
    bvecs += [bu, f(bk), f(bv), f(bp)]
    bcol = np.concatenate([_bias_col(v) for v in bvecs], axis=1)

    in_maps = []
    for b in range(B):
        xb = x[b].T  # [64, T]
        xcm = np.zeros((128, CT), np.float32)
        xcm[0:64, PAD:] = xb
        xcm[64:128, PAD:] = np.pad(xb, ((0, 0), (1, 0)))[:, :T]   # x(t-1)
        in_maps.append({
            "xcm": np.ascontiguousarray(xcm).astype(bf16),
            "ones": np.ones((1, 128), np.float32),
            "zpad": np.zeros((128, PAD), bf16),
            "w0c1": w0c1,
            "wres": wres_p,
            "wmain": wmain,
            "wu": wu_p,
            "wpv": wpv_p,
            "wp": wp_p,
            "bcol": bcol,
        })
    return in_maps


def kernel(**inputs):
    from concourse import bass_utils
    nc = get_program()
    in_maps = make_in_maps(**inputs)
    res = bass_utils.run_bass_kernel_spmd(nc, in_maps, core_ids=list(range(B)))
    out = np.stack([res.results[b]["out"] for b in range(B)], axis=0)
    return out.astype(np.float32)
